# revision 23
# baseline (speedup 1.0000x reference)
"""3-layer GCN (N=50000, E=1.6M + self-loops) on 8 TRN2 NeuronCores.

Node/data-parallel (per sharding hint): core c owns rows [6250c, 6250(c+1)),
padded to 6272. Per layer: local transform (feature-major on PE), deg^-1/2
prescale, PE transpose to node-major, and TWO AllGathers build replicated
bf16 feature tables (halves A/B, int16-addressable). Aggregation runs in two
passes per 128-dst tile: dma_gather (4 SWDGE queues) pulls the A-half edge
slots, PE computes aggT += M.T @ S (S = one-hot dst-row matrix, built once on
DVE via is_equal and spilled/reloaded through DRAM), partial sums park in an
SBUF accumulator while pass B overlaps the B-half AllGather. Epilogue applies
deg^-1/2 postscale + bias + ReLU; layer 3 aggregates first (associativity).
The next layer's transform/staging/AllGather-A are emitted interleaved with
pass B so collectives hide behind gather work.
"""
import os

import numpy as np
import ml_dtypes

import concourse.bacc as bacc
import concourse.bass as bass
import concourse.mybir as mybir
import concourse.tile as tile
from concourse.bass_utils import run_bass_kernel_spmd
from concourse.library_config import mlp
from concourse.masks import make_identity

N = 50000
NCORES = 8
PER = 6250
PAD = 6272            # per-core padded node count (49 * 128)
NTILE = PAD // 128    # 49
LOCA = 3200           # local split: half A = [0, 3200), half B = [3200, 6272)
LOCB = PAD - LOCA     # 3072
ROWSA = LOCA * NCORES  # 25600 (int16-safe)
ROWSB = LOCB * NCORES  # 24576
K1 = 12
D0, D1, D2, D3 = 1433, 100, 50, 7
ELEM = 128            # bf16 elements per table row (256B)
CALL_MAX = 1024       # SWDGE ring capacity per dma_gather call
BF16 = ml_dtypes.bfloat16

LAST_EXEC_NS = None
_CACHE = {}


def _ceil128(x):
    return (np.asarray(x) + 127) // 128 * 128


def _wrap_idx(a):
    w = a.reshape(-1, 16).T
    return np.tile(w, (8, 1)).astype(np.int16)


def _wrap_slot(a, dtype):
    return np.ascontiguousarray(a.reshape(-1, 128).T).astype(dtype)


def _prep_graph(edge_index):
    src = np.asarray(edge_index[0], np.int64)
    dst = np.asarray(edge_index[1], np.int64)
    loops = np.arange(N, dtype=np.int64)
    # deg counts self-loops (reference adds them); the loops themselves are
    # NOT gathered -- their ds[v]^2*H[v] term is added densely in pass A.
    deg = np.bincount(np.concatenate([dst, loops]), minlength=N).astype(np.float64)
    ds = (1.0 / np.sqrt(deg)).astype(np.float32)
    srcA = src
    dstA = dst

    core = dstA // PER
    local = dstA - core * PER
    tilep = local >> 7
    drow = (local & 127).astype(np.int64)
    score = srcA // PER
    slocal = srcA - score * PER
    half = (slocal >= LOCA).astype(np.int64)
    idx16 = np.where(half == 0, score * LOCA + slocal, score * LOCB + (slocal - LOCA))

    key = ((core * NTILE + tilep) * 2 + half).astype(np.int64)
    order = np.argsort(key, kind="stable")
    key_s = key[order]
    idx_s = idx16[order]
    drow_s = drow[order]

    ngroups = NCORES * NTILE * 2
    counts = np.bincount(key_s, minlength=ngroups).reshape(NCORES, NTILE, 2)
    runlen = _ceil128(counts.max(axis=0))
    runlen = np.maximum(runlen, 128)
    starts = np.zeros(ngroups + 1, np.int64)
    np.cumsum(np.bincount(key_s, minlength=ngroups), out=starts[1:])

    # slot layout: ALL A-runs (tile-major), then ALL B-runs
    stot = int(runlen.sum())
    offA = np.concatenate([[0], np.cumsum(runlen[:, 0])])
    offB = np.concatenate([[0], np.cumsum(runlen[:, 1])]) + offA[-1]
    idx_pad = np.zeros((NCORES, stot), np.int64)
    drow_pad = np.full((NCORES, stot), 300, np.int64)
    for t in range(NTILE):
        for h in range(2):
            o0 = int(offA[t] if h == 0 else offB[t])
            for c in range(NCORES):
                g = (c * NTILE + t) * 2 + h
                n = int(starts[g + 1] - starts[g])
                idx_pad[c, o0 : o0 + n] = idx_s[starts[g] : starts[g + 1]]
                drow_pad[c, o0 : o0 + n] = drow_s[starts[g] : starts[g + 1]]

    idxw = np.stack([_wrap_idx(idx_pad[c]) for c in range(NCORES)])
    droww = np.stack(
        [_wrap_slot(drow_pad[c].astype(np.float32), BF16) for c in range(NCORES)]
    )
    return ds, runlen, idxw, droww


def _build(runlen, nchunk_max):
    dt = mybir.dt
    stot = int(runlen.sum())
    nchunk_tot = stot // 128
    offA = np.concatenate([[0], np.cumsum(runlen[:, 0])])
    offB = np.concatenate([[0], np.cumsum(runlen[:, 1])]) + offA[-1]

    nc = bacc.Bacc("TRN2", target_bir_lowering=False, debug=False, num_swdge_queues=4)
    xT = nc.dram_tensor("xT", [K1 * 128, PAD], dt.bfloat16, kind="ExternalInput")
    w1 = nc.dram_tensor("w1", [K1, 128, D1], dt.bfloat16, kind="ExternalInput")
    w2 = nc.dram_tensor("w2", [D1, D2], dt.float32, kind="ExternalInput")
    w3 = nc.dram_tensor("w3", [D2, D3], dt.float32, kind="ExternalInput")
    b1 = nc.dram_tensor("b1", [128, 1], dt.float32, kind="ExternalInput")
    b2 = nc.dram_tensor("b2", [128, 1], dt.float32, kind="ExternalInput")
    b3 = nc.dram_tensor("b3", [128, 1], dt.float32, kind="ExternalInput")
    dsrep = nc.dram_tensor("dsrep", [128, PAD], dt.float32, kind="ExternalInput")
    iota = nc.dram_tensor("iota", [128, 128], dt.bfloat16, kind="ExternalInput")
    idxs = nc.dram_tensor("idxs", [128, stot // 16], dt.int16, kind="ExternalInput")
    dstrow = nc.dram_tensor("dstrow", [128, nchunk_tot], dt.bfloat16, kind="ExternalInput")
    out_d = nc.dram_tensor("out", [D3, PAD], dt.float32, kind="ExternalOutput")

    s_spill = nc.dram_tensor("s_spill", [128, nchunk_tot * 128], dt.bfloat16)
    ag_inA = [nc.dram_tensor(f"ag_inA{i}", [LOCA, ELEM], dt.bfloat16) for i in range(3)]
    ag_inB = [nc.dram_tensor(f"ag_inB{i}", [LOCB, ELEM], dt.bfloat16) for i in range(3)]
    tabA = [
        nc.dram_tensor(f"tabA{i}", [ROWSA, ELEM], dt.bfloat16, addr_space="Shared")
        for i in range(3)
    ]
    tabB = [
        nc.dram_tensor(f"tabB{i}", [ROWSB, ELEM], dt.bfloat16, addr_space="Shared")
        for i in range(3)
    ]

    # v-groups (128-multiples, not straddling LOCA)
    groups = [(g * 512, 512) for g in range(6)] + [(3072, 128)] + [
        (LOCA + g * 512, 512) for g in range(6)
    ]
    groupsA = groups[:7]
    groupsB = groups[7:]

    with tile.TileContext(nc) as tc:
        with (
            tc.tile_pool(name="const", bufs=1) as constp,
            tc.tile_pool(name="big", bufs=1) as bigp,
            tc.tile_pool(name="slab", bufs=2) as slabp,
            tc.tile_pool(name="stage", bufs=3) as stagep,
            tc.tile_pool(name="m", bufs=7) as mp,
            tc.tile_pool(name="s", bufs=2) as sp,
            tc.tile_pool(name="eptmp", bufs=2) as epp,
            tc.tile_pool(name="psA", bufs=2, space="PSUM") as psA,
            tc.tile_pool(name="psB", bufs=2, space="PSUM") as psB,
            tc.tile_pool(name="psC", bufs=2, space="PSUM") as psC,
        ):
            nc.gpsimd.load_library(mlp)
            ident = constp.tile([128, 128], dt.float32)
            make_identity(nc, ident[:])
            w1_sb = constp.tile([128, K1, D1], dt.bfloat16)
            nc.sync.dma_start(
                w1_sb[:],
                bass.AP(w1.ap().tensor, 0, [[D1, 128], [128 * D1, K1], [1, D1]]),
            )
            w2_sb = constp.tile([128, D2], dt.float32)
            nc.sync.dma_start(w2_sb[0:D1, :], w2[:, :])
            w3_sb = constp.tile([128, D3], dt.float32)
            nc.sync.dma_start(w3_sb[0:D2, :], w3[:, :])
            b_sb = []
            for bt in (b1, b2, b3):
                b = constp.tile([128, 1], dt.float32)
                nc.sync.dma_start(b[:], bt[:, :])
                b_sb.append(b)
            dsr = constp.tile([128, PAD], dt.float32)
            nc.sync.dma_start(dsr[:], dsrep[:, :])
            iot = constp.tile([128, 128], dt.bfloat16)
            nc.sync.dma_start(iot[:], iota[:, :])
            idx_sb = constp.tile([128, stot // 16], dt.int16)
            nc.sync.dma_start(idx_sb[:], idxs[:, :])
            drow_sb = constp.tile([128, nchunk_tot], dt.bfloat16)
            nc.sync.dma_start(drow_sb[:], dstrow[:, :])

            out1T = bigp.tile([128, PAD], dt.float32)
            out2T = bigp.tile([128, PAD], dt.float32)
            out3T = bigp.tile([128, PAD], dt.float32)
            acc = bigp.tile([128, PAD], dt.bfloat16)  # pass-A partial sums
            pre_own = bigp.tile([128, PAD], dt.bfloat16)  # own prescaled rows^T

            qn = [0]
            ni_regs = {}

            def ni_reg(ni):
                if ni not in ni_regs:
                    ni_regs[ni] = nc.gpsimd.to_reg(ni)
                return ni_regs[ni]

            def stage_table(li, src_big, du, g0, w, src_off=None):
                o = g0 if src_off is None else src_off
                ts = epp.tile([128, 512], dt.float32, tag="ts")
                nc.vector.tensor_tensor(
                    ts[0:du, 0:w], src_big[0:du, o : o + w],
                    dsr[0:du, g0 : g0 + w], mybir.AluOpType.mult,
                )
                nc.vector.tensor_copy(pre_own[0:du, g0 : g0 + w], ts[0:du, 0:w])
                agd = ag_inA[li] if g0 < LOCA else ag_inB[li]
                r0 = g0 if g0 < LOCA else g0 - LOCA
                for s in range(w // 128):
                    pt = psC.tile([128, 128], dt.float32, tag="pt")
                    nc.tensor.transpose(
                        pt[:], ts[0:du, s * 128 : (s + 1) * 128], ident[0:du, :]
                    )
                    st = stagep.tile([128, ELEM], dt.bfloat16, tag="st")
                    nc.vector.tensor_copy(st[:], pt[:])
                    nc.sync.dma_start(agd[r0 + s * 128 : r0 + (s + 1) * 128, :], st[:])

            def allgather(li, h):
                src = ag_inA[li] if h == 0 else ag_inB[li]
                dst = tabA[li] if h == 0 else tabB[li]
                nc.gpsimd.collective_compute(
                    "AllGather",
                    mybir.AluOpType.bypass,
                    replica_groups=[list(range(NCORES))],
                    ins=[src.ap().opt()],
                    outs=[dst.ap().opt()],
                )

            WCH = CALL_MAX // 128  # chunks per gather window

            def seg_s(t, h, li, m, nch, j0):
                """S chunks for (tile, half) into s tile at chunk offset j0."""
                chunk0 = int(offA[t] if h == 0 else offB[t]) // 128
                s_t = sp.tile([128, nchunk_max, 128], dt.bfloat16, tag="s")
                sd = s_spill[:, chunk0 * 128 : (chunk0 + nch) * 128]
                if li == 0:
                    da = drow_sb[:, chunk0 : chunk0 + nch].to_broadcast([128, nch, 128])
                    ia = iot[:, :]
                    ia = bass.AP(ia.tensor, ia.offset, [ia.ap[0], [0, nch], ia.ap[1]])
                    nc.vector.tensor_tensor(
                        s_t[:, 0:nch, :], da, ia, mybir.AluOpType.is_equal
                    )
                    nc.sync.dma_start(sd, s_t[:, 0:nch, :])
                else:
                    nc.sync.dma_start(s_t[:, 0:nch, :], sd)
                return s_t

            def pass_h(li, h, epilogue, post_tile=None):
                src = (tabA[li] if h == 0 else tabB[li])[:, :]
                base = 0 if h == 0 else int(offA[-1])
                total = int(runlen[:, h].sum())
                wins = {}

                def get_win(w):
                    if w not in wins:
                        ni = min(CALL_MAX, total - w * CALL_MAX)
                        mw = mp.tile([128, WCH, ELEM], dt.bfloat16, tag="m")
                        s0 = base + w * CALL_MAX
                        nc.gpsimd.dma_gather(
                            mw[:, 0 : ni // 128, :],
                            src,
                            idx_sb[:, s0 // 16 : (s0 + ni) // 16],
                            ni,
                            ni_reg(ni),
                            ELEM,
                            queue_num=qn[0] % 4,
                        )
                        qn[0] += 1
                        wins[w] = mw
                    return wins[w]

                for t in range(NTILE):
                    rl = int(runlen[t, h])
                    nch = rl // 128
                    slot0 = int(offA[t] if h == 0 else offB[t])
                    s_t = seg_s(t, h, li, None, nch, 0)
                    ps = psA.tile([128, 128], dt.float32, tag="agg")
                    for j in range(nch):
                        rel = (slot0 - base) // 128 + j
                        mw = get_win(rel // WCH)
                        nc.tensor.matmul(
                            ps[:], mw[:, rel % WCH, :], s_t[:, j, :],
                            start=(j == 0), stop=(j == nch - 1),
                        )
                    epilogue(t, ps)
                    if post_tile is not None:
                        post_tile(t)

            def epA(t, ps):
                # park pass-A sum + the self-loop (diagonal) term
                nc.vector.tensor_tensor(
                    acc[:, t * 128 : (t + 1) * 128], ps[:],
                    pre_own[:, t * 128 : (t + 1) * 128], mybir.AluOpType.add,
                )

            def mk_epB(li, du, out_big, bias, final3=False):
                def ep(t, ps):
                    sc = epp.tile([128, 128], dt.float32, tag="sc")
                    # total = psB_pass + accA (bf16 partials)
                    nc.vector.tensor_tensor(
                        sc[0:du, :], ps[0:du, :],
                        acc[0:du, t * 128 : (t + 1) * 128], mybir.AluOpType.add,
                    )
                    nc.vector.tensor_tensor(
                        sc[0:du, :], sc[0:du, :],
                        dsr[0:du, t * 128 : (t + 1) * 128], mybir.AluOpType.mult,
                    )
                    if final3:
                        po = psB.tile([128, 128], dt.float32, tag="mm3")
                        nc.tensor.matmul(po[0:D3, :], w3_sb[0:D2, :], sc[0:du, :])
                        nc.scalar.activation(
                            out_big[0:D3, t * 128 : (t + 1) * 128], po[0:D3, :],
                            mybir.ActivationFunctionType.Relu, bias=bias[0:D3, :],
                        )
                    else:
                        nc.scalar.activation(
                            out_big[0:du, t * 128 : (t + 1) * 128], sc[0:du, :],
                            mybir.ActivationFunctionType.Relu, bias=bias[0:du, :],
                        )
                return ep

            # ---------- Layer 1 transform + staged AGs ----------
            def l1_group(g0, w):
                slab = slabp.tile([128, K1, 512], dt.bfloat16, tag="slab")
                nc.sync.dma_start(
                    slab[:, :, 0:w],
                    bass.AP(xT.ap().tensor, g0, [[PAD, 128], [128 * PAD, K1], [1, w]]),
                )
                ph = psB.tile([128, 512], dt.float32, tag="mm")
                for k in range(K1):
                    nc.tensor.matmul(
                        ph[0:D1, 0:w], w1_sb[:, k, :], slab[:, k, 0:w],
                        start=(k == 0), stop=(k == K1 - 1),
                    )
                stage_table(0, ph, D1, g0, w, src_off=0)

            for g0, w in groupsA:
                l1_group(g0, w)
            allgather(0, 0)
            for g0, w in groupsB:
                l1_group(g0, w)
            allgather(0, 1)

            # interleaved emission of next-layer transform during pass B
            def mk_post(emit_group, li_next):
                done = [0]
                allgroups = groups

                def post(t):
                    # after tile t, columns up to (t+1)*128 of the source are ready
                    ready = (t + 1) * 128
                    while done[0] < len(allgroups):
                        g0, w = allgroups[done[0]]
                        if g0 + w <= ready:
                            emit_group(g0, w)
                            done[0] += 1
                            if done[0] == 7:
                                allgather(li_next, 0)
                        else:
                            break
                    if t == NTILE - 1:
                        while done[0] < len(allgroups):
                            g0, w = allgroups[done[0]]
                            emit_group(g0, w)
                            done[0] += 1
                            if done[0] == 7:
                                allgather(li_next, 0)
                        allgather(li_next, 1)

                return post

            # ---------- Layer 1 aggregation ----------
            def l2_group(g0, w):
                ph = psB.tile([128, 512], dt.float32, tag="mm")
                nc.tensor.matmul(ph[0:D2, 0:w], w2_sb[0:D1, :], out1T[0:D1, g0 : g0 + w])
                stage_table(1, ph, D2, g0, w, src_off=0)

            pass_h(0, 0, epA)
            pass_h(0, 1, mk_epB(0, D1, out1T, b_sb[0]), post_tile=mk_post(l2_group, 1))

            # ---------- Layer 2 aggregation ----------
            def l3_group(g0, w):
                stage_table(2, out2T, D2, g0, w)

            pass_h(1, 0, epA)
            pass_h(1, 1, mk_epB(1, D2, out2T, b_sb[1]), post_tile=mk_post(l3_group, 2))

            # ---------- Layer 3 aggregation (aggregate-first) ----------
            pass_h(2, 0, epA)
            pass_h(2, 1, mk_epB(2, D2, out3T, b_sb[2], final3=True))

            nc.sync.dma_start(out_d[:, :], out3T[0:D3, :])

    nc.compile()
    return nc


def kernel(**inputs):
    global LAST_EXEC_NS
    x = np.asarray(inputs["x"], np.float32)
    ei = np.asarray(inputs["edge_index"])
    W = [np.asarray(inputs[f"W{i}"], np.float32) for i in (1, 2, 3)]
    b = [np.asarray(inputs[f"b{i}"], np.float32) for i in (1, 2, 3)]

    ds, runlen, idxw, droww = _prep_graph(ei)
    nchunk_max = int((runlen // 128).max())
    key = (tuple(runlen.ravel().tolist()), nchunk_max)
    if key not in _CACHE:
        _CACHE[key] = _build(runlen, nchunk_max)
    nc = _CACHE[key]

    w1p = np.zeros((K1 * 128, D1), np.float32)
    w1p[:D0] = W[0]
    w1p = np.ascontiguousarray(w1p.reshape(K1, 128, D1)).astype(BF16)
    bp = []
    for i, d in enumerate((D1, D2, D3)):
        a = np.zeros((128, 1), np.float32)
        a[:d, 0] = b[i]
        bp.append(a)
    iota = np.tile(np.arange(128, dtype=np.float32), (128, 1)).astype(BF16)

    in_maps = []
    for c in range(NCORES):
        sl = slice(c * PER, (c + 1) * PER)
        xTp = np.zeros((K1 * 128, PAD), BF16)
        xTp[:D0, :PER] = x[sl].T.astype(BF16)
        dsl = np.zeros(PAD, np.float32)
        dsl[:PER] = ds[sl]
        in_maps.append(
            {
                "xT": xTp,
                "w1": w1p,
                "w2": W[1],
                "w3": W[2],
                "b1": bp[0],
                "b2": bp[1],
                "b3": bp[2],
                "dsrep": np.ascontiguousarray(np.broadcast_to(dsl, (128, PAD))),
                "iota": iota,
                "idxs": idxw[c],
                "dstrow": droww[c],
            }
        )

    trace = bool(int(os.environ.get("KERNEL_TRACE", "0")))
    if trace:
        try:
            import trnprof  # noqa: F401  (dev-only profiling shim)
        except ImportError:
            trace = False

    res = run_bass_kernel_spmd(nc, in_maps, list(range(NCORES)), trace=trace)
    LAST_EXEC_NS = res.exec_time_ns

    out = np.empty((N, D3), np.float32)
    for c in range(NCORES):
        out[c * PER : (c + 1) * PER] = res.results[c]["out"][:, :PER].T
    return out


# revision 25
# speedup vs baseline: 1.0346x; 1.0346x over previous
"""3-layer GCN (N=50000, E=1.6M + self-loops) on 8 TRN2 NeuronCores.

Node/data-parallel (per sharding hint): core c owns rows [6250c, 6250(c+1)),
padded to 6272. Per layer: local transform (feature-major on PE), deg^-1/2
prescale, PE transpose to node-major, and TWO AllGathers build replicated
bf16 feature tables (halves A/B, int16-addressable). Aggregation runs in two
passes per 128-dst tile: dma_gather (4 SWDGE queues) pulls the A-half edge
slots, PE computes aggT += M.T @ S (S = one-hot dst-row matrix, built once on
DVE via is_equal and spilled/reloaded through DRAM), partial sums park in an
SBUF accumulator while pass B overlaps the B-half AllGather. Epilogue applies
deg^-1/2 postscale + bias + ReLU; layer 3 aggregates first (associativity).
The next layer's transform/staging/AllGather-A are emitted interleaved with
pass B so collectives hide behind gather work.
"""
import os

import numpy as np
import ml_dtypes

import concourse.bacc as bacc
import concourse.bass as bass
import concourse.mybir as mybir
import concourse.tile as tile
from concourse.bass_utils import run_bass_kernel_spmd
from concourse.library_config import mlp
from concourse.masks import make_identity

N = 50000
NCORES = 8
PER = 6250
PAD = 6272            # per-core padded node count (49 * 128)
NTILE = PAD // 128    # 49
LOCA = 3200           # local split: half A = [0, 3200), half B = [3200, 6272)
LOCB = PAD - LOCA     # 3072
ROWSA = LOCA * NCORES  # 25600 (int16-safe)
ROWSB = LOCB * NCORES  # 24576
K1 = 12
D0, D1, D2, D3 = 1433, 100, 50, 7
ELEM = 128            # bf16 elements per table row (256B)
CALL_MAX = 1024       # SWDGE ring capacity per dma_gather call
BF16 = ml_dtypes.bfloat16

LAST_EXEC_NS = None
_CACHE = {}


def _ceil128(x):
    return (np.asarray(x) + 127) // 128 * 128


def _wrap_idx(a):
    w = a.reshape(-1, 16).T
    return np.tile(w, (8, 1)).astype(np.int16)


def _wrap_slot(a, dtype):
    return np.ascontiguousarray(a.reshape(-1, 128).T).astype(dtype)


def _prep_graph(edge_index):
    src = np.asarray(edge_index[0], np.int64)
    dst = np.asarray(edge_index[1], np.int64)
    loops = np.arange(N, dtype=np.int64)
    # deg counts self-loops (reference adds them); the loops themselves are
    # NOT gathered -- their ds[v]^2*H[v] term is added densely in pass A.
    deg = np.bincount(np.concatenate([dst, loops]), minlength=N).astype(np.float64)
    ds = (1.0 / np.sqrt(deg)).astype(np.float32)
    srcA = src
    dstA = dst

    core = dstA // PER
    local = dstA - core * PER
    tilep = local >> 7
    drow = (local & 127).astype(np.int64)
    score = srcA // PER
    slocal = srcA - score * PER
    half = (slocal >= LOCA).astype(np.int64)
    idx16 = np.where(half == 0, score * LOCA + slocal, score * LOCB + (slocal - LOCA))

    key = ((core * NTILE + tilep) * 2 + half).astype(np.int64)
    order = np.argsort(key, kind="stable")
    key_s = key[order]
    idx_s = idx16[order]
    drow_s = drow[order]

    ngroups = NCORES * NTILE * 2
    counts = np.bincount(key_s, minlength=ngroups).reshape(NCORES, NTILE, 2)
    runlen = _ceil128(counts.max(axis=0))
    runlen = np.maximum(runlen, 128)
    starts = np.zeros(ngroups + 1, np.int64)
    np.cumsum(np.bincount(key_s, minlength=ngroups), out=starts[1:])

    # slot layout: ALL A-runs (tile-major), then ALL B-runs
    stot = int(runlen.sum())
    offA = np.concatenate([[0], np.cumsum(runlen[:, 0])])
    offB = np.concatenate([[0], np.cumsum(runlen[:, 1])]) + offA[-1]
    idx_pad = np.zeros((NCORES, stot), np.int64)
    drow_pad = np.full((NCORES, stot), 300, np.int64)
    for t in range(NTILE):
        for h in range(2):
            o0 = int(offA[t] if h == 0 else offB[t])
            for c in range(NCORES):
                g = (c * NTILE + t) * 2 + h
                n = int(starts[g + 1] - starts[g])
                idx_pad[c, o0 : o0 + n] = idx_s[starts[g] : starts[g + 1]]
                drow_pad[c, o0 : o0 + n] = drow_s[starts[g] : starts[g + 1]]

    idxw = np.stack([_wrap_idx(idx_pad[c]) for c in range(NCORES)])
    droww = np.stack(
        [_wrap_slot(drow_pad[c].astype(np.float32), BF16) for c in range(NCORES)]
    )
    return ds, runlen, idxw, droww


def _build(runlen, nchunk_max):
    dt = mybir.dt
    stot = int(runlen.sum())
    nchunk_tot = stot // 128
    offA = np.concatenate([[0], np.cumsum(runlen[:, 0])])
    offB = np.concatenate([[0], np.cumsum(runlen[:, 1])]) + offA[-1]

    nc = bacc.Bacc("TRN2", target_bir_lowering=False, debug=False, num_swdge_queues=4)
    xT = nc.dram_tensor("xT", [K1 * 128, PAD], dt.bfloat16, kind="ExternalInput")
    w1 = nc.dram_tensor("w1", [K1, 128, D1], dt.bfloat16, kind="ExternalInput")
    w2 = nc.dram_tensor("w2", [D1, D2], dt.float32, kind="ExternalInput")
    w3 = nc.dram_tensor("w3", [D2, D3], dt.float32, kind="ExternalInput")
    b1 = nc.dram_tensor("b1", [128, 1], dt.float32, kind="ExternalInput")
    b2 = nc.dram_tensor("b2", [128, 1], dt.float32, kind="ExternalInput")
    b3 = nc.dram_tensor("b3", [128, 1], dt.float32, kind="ExternalInput")
    dsrep = nc.dram_tensor("dsrep", [128, PAD], dt.float32, kind="ExternalInput")
    iota = nc.dram_tensor("iota", [128, 128], dt.bfloat16, kind="ExternalInput")
    idxs = nc.dram_tensor("idxs", [128, stot // 16], dt.int16, kind="ExternalInput")
    dstrow = nc.dram_tensor("dstrow", [128, nchunk_tot], dt.bfloat16, kind="ExternalInput")
    out_d = nc.dram_tensor("out", [D3, PAD], dt.float32, kind="ExternalOutput")

    s_spill = nc.dram_tensor("s_spill", [128, nchunk_tot * 128], dt.bfloat16)
    ag_inA = [nc.dram_tensor(f"ag_inA{i}", [LOCA, ELEM], dt.bfloat16) for i in range(3)]
    ag_inB = [nc.dram_tensor(f"ag_inB{i}", [LOCB, ELEM], dt.bfloat16) for i in range(3)]
    tabA = [
        nc.dram_tensor(f"tabA{i}", [ROWSA, ELEM], dt.bfloat16, addr_space="Shared")
        for i in range(3)
    ]
    tabB = [
        nc.dram_tensor(f"tabB{i}", [ROWSB, ELEM], dt.bfloat16, addr_space="Shared")
        for i in range(3)
    ]

    # v-groups (128-multiples, not straddling LOCA)
    groups = [(g * 512, 512) for g in range(6)] + [(3072, 128)] + [
        (LOCA + g * 512, 512) for g in range(6)
    ]
    groupsA = groups[:7]
    groupsB = groups[7:]

    with tile.TileContext(nc) as tc:
        with (
            tc.tile_pool(name="const", bufs=1) as constp,
            tc.tile_pool(name="big", bufs=1) as bigp,
            tc.tile_pool(name="slab", bufs=1) as slabp,
            tc.tile_pool(name="stage", bufs=3) as stagep,
            tc.tile_pool(name="m", bufs=8) as mp,
            tc.tile_pool(name="s", bufs=4) as sp,
            tc.tile_pool(name="eptmp", bufs=2) as epp,
            tc.tile_pool(name="psA", bufs=2, space="PSUM") as psA,
            tc.tile_pool(name="psB", bufs=2, space="PSUM") as psB,
            tc.tile_pool(name="psC", bufs=2, space="PSUM") as psC,
        ):
            nc.gpsimd.load_library(mlp)
            ident = constp.tile([128, 128], dt.float32)
            make_identity(nc, ident[:])
            w1_sb = constp.tile([128, K1, D1], dt.bfloat16)
            nc.sync.dma_start(
                w1_sb[:],
                bass.AP(w1.ap().tensor, 0, [[D1, 128], [128 * D1, K1], [1, D1]]),
            )
            w2_sb = constp.tile([128, D2], dt.float32)
            nc.sync.dma_start(w2_sb[0:D1, :], w2[:, :])
            w3_sb = constp.tile([128, D3], dt.float32)
            nc.sync.dma_start(w3_sb[0:D2, :], w3[:, :])
            b_sb = []
            for bt in (b1, b2, b3):
                b = constp.tile([128, 1], dt.float32)
                nc.sync.dma_start(b[:], bt[:, :])
                b_sb.append(b)
            dsr = constp.tile([128, PAD], dt.float32)
            nc.sync.dma_start(dsr[:], dsrep[:, :])
            iot = constp.tile([128, 128], dt.bfloat16)
            nc.sync.dma_start(iot[:], iota[:, :])
            idx_sb = constp.tile([128, stot // 16], dt.int16)
            nc.sync.dma_start(idx_sb[:], idxs[:, :])
            drow_sb = constp.tile([128, nchunk_tot], dt.bfloat16)
            nc.sync.dma_start(drow_sb[:], dstrow[:, :])

            out1T = bigp.tile([128, PAD], dt.float32)
            out2T = bigp.tile([128, PAD], dt.float32)
            out3T = bigp.tile([128, PAD], dt.float32)
            acc = bigp.tile([128, PAD], dt.bfloat16)  # pass-A partial sums
            pre_own = bigp.tile([128, PAD], dt.bfloat16)  # own prescaled rows^T

            qn = [0]
            ni_regs = {}

            def ni_reg(ni):
                if ni not in ni_regs:
                    ni_regs[ni] = nc.gpsimd.to_reg(ni)
                return ni_regs[ni]

            def stage_table(li, src_big, du, g0, w, src_off=None):
                o = g0 if src_off is None else src_off
                ts = epp.tile([128, 512], dt.float32, tag="ts")
                nc.vector.tensor_tensor(
                    ts[0:du, 0:w], src_big[0:du, o : o + w],
                    dsr[0:du, g0 : g0 + w], mybir.AluOpType.mult,
                )
                nc.vector.tensor_copy(pre_own[0:du, g0 : g0 + w], ts[0:du, 0:w])
                agd = ag_inA[li] if g0 < LOCA else ag_inB[li]
                r0 = g0 if g0 < LOCA else g0 - LOCA
                for s in range(w // 128):
                    pt = psC.tile([128, 128], dt.float32, tag="pt")
                    nc.tensor.transpose(
                        pt[:], ts[0:du, s * 128 : (s + 1) * 128], ident[0:du, :]
                    )
                    st = stagep.tile([128, ELEM], dt.bfloat16, tag="st")
                    nc.vector.tensor_copy(st[:], pt[:])
                    nc.sync.dma_start(agd[r0 + s * 128 : r0 + (s + 1) * 128, :], st[:])

            def allgather(li, h):
                src = ag_inA[li] if h == 0 else ag_inB[li]
                dst = tabA[li] if h == 0 else tabB[li]
                nc.gpsimd.collective_compute(
                    "AllGather",
                    mybir.AluOpType.bypass,
                    replica_groups=[list(range(NCORES))],
                    ins=[src.ap().opt()],
                    outs=[dst.ap().opt()],
                )

            WCH = CALL_MAX // 128  # chunks per gather window

            def seg_s(t, h, li, m, nch, j0):
                """S chunks for (tile, half) into s tile at chunk offset j0."""
                chunk0 = int(offA[t] if h == 0 else offB[t]) // 128
                s_t = sp.tile([128, nchunk_max, 128], dt.bfloat16, tag="s")
                sd = s_spill[:, chunk0 * 128 : (chunk0 + nch) * 128]
                if li == 0:
                    da = drow_sb[:, chunk0 : chunk0 + nch].to_broadcast([128, nch, 128])
                    ia = iot[:, :]
                    ia = bass.AP(ia.tensor, ia.offset, [ia.ap[0], [0, nch], ia.ap[1]])
                    nc.vector.tensor_tensor(
                        s_t[:, 0:nch, :], da, ia, mybir.AluOpType.is_equal
                    )
                    nc.sync.dma_start(sd, s_t[:, 0:nch, :])
                else:
                    nc.sync.dma_start(s_t[:, 0:nch, :], sd)
                return s_t

            def pass_h(li, h, epilogue, post_tile=None):
                src = (tabA[li] if h == 0 else tabB[li])[:, :]
                base = 0 if h == 0 else int(offA[-1])
                total = int(runlen[:, h].sum())
                wins = {}

                def get_win(w):
                    if w not in wins:
                        ni = min(CALL_MAX, total - w * CALL_MAX)
                        mw = mp.tile([128, WCH, ELEM], dt.bfloat16, tag="m")
                        s0 = base + w * CALL_MAX
                        nc.gpsimd.dma_gather(
                            mw[:, 0 : ni // 128, :],
                            src,
                            idx_sb[:, s0 // 16 : (s0 + ni) // 16],
                            ni,
                            ni,
                            ELEM,
                            queue_num=qn[0] % 4,
                        )
                        qn[0] += 1
                        wins[w] = mw
                    return wins[w]

                for t in range(NTILE):
                    rl = int(runlen[t, h])
                    nch = rl // 128
                    slot0 = int(offA[t] if h == 0 else offB[t])
                    s_t = seg_s(t, h, li, None, nch, 0)
                    ps = psA.tile([128, 128], dt.float32, tag="agg")
                    for j in range(nch):
                        rel = (slot0 - base) // 128 + j
                        mw = get_win(rel // WCH)
                        nc.tensor.matmul(
                            ps[:], mw[:, rel % WCH, :], s_t[:, j, :],
                            start=(j == 0), stop=(j == nch - 1),
                        )
                    epilogue(t, ps)
                    if post_tile is not None:
                        post_tile(t)

            def epA(t, ps):
                # park pass-A sum + the self-loop (diagonal) term
                nc.vector.tensor_tensor(
                    acc[:, t * 128 : (t + 1) * 128], ps[:],
                    pre_own[:, t * 128 : (t + 1) * 128], mybir.AluOpType.add,
                )

            def mk_epB(li, du, out_big, bias, final3=False):
                def ep(t, ps):
                    sc = epp.tile([128, 128], dt.float32, tag="sc")
                    # total = psB_pass + accA (bf16 partials)
                    nc.vector.tensor_tensor(
                        sc[0:du, :], ps[0:du, :],
                        acc[0:du, t * 128 : (t + 1) * 128], mybir.AluOpType.add,
                    )
                    nc.vector.tensor_tensor(
                        sc[0:du, :], sc[0:du, :],
                        dsr[0:du, t * 128 : (t + 1) * 128], mybir.AluOpType.mult,
                    )
                    if final3:
                        po = psB.tile([128, 128], dt.float32, tag="mm3")
                        nc.tensor.matmul(po[0:D3, :], w3_sb[0:D2, :], sc[0:du, :])
                        nc.scalar.activation(
                            out_big[0:D3, t * 128 : (t + 1) * 128], po[0:D3, :],
                            mybir.ActivationFunctionType.Relu, bias=bias[0:D3, :],
                        )
                    else:
                        nc.scalar.activation(
                            out_big[0:du, t * 128 : (t + 1) * 128], sc[0:du, :],
                            mybir.ActivationFunctionType.Relu, bias=bias[0:du, :],
                        )
                return ep

            # ---------- Layer 1 transform + staged AGs ----------
            def l1_group(g0, w):
                slab = slabp.tile([128, K1, 512], dt.bfloat16, tag="slab")
                nc.sync.dma_start(
                    slab[:, :, 0:w],
                    bass.AP(xT.ap().tensor, g0, [[PAD, 128], [128 * PAD, K1], [1, w]]),
                )
                ph = psB.tile([128, 512], dt.float32, tag="mm")
                for k in range(K1):
                    nc.tensor.matmul(
                        ph[0:D1, 0:w], w1_sb[:, k, :], slab[:, k, 0:w],
                        start=(k == 0), stop=(k == K1 - 1),
                    )
                stage_table(0, ph, D1, g0, w, src_off=0)

            for g0, w in groupsA:
                l1_group(g0, w)
            allgather(0, 0)
            for g0, w in groupsB:
                l1_group(g0, w)
            allgather(0, 1)

            # interleaved emission of next-layer transform during pass B
            def mk_post(emit_group, li_next):
                done = [0]
                allgroups = groups

                def post(t):
                    # after tile t, columns up to (t+1)*128 of the source are ready
                    ready = (t + 1) * 128
                    while done[0] < len(allgroups):
                        g0, w = allgroups[done[0]]
                        if g0 + w <= ready:
                            emit_group(g0, w)
                            done[0] += 1
                            if done[0] == 7:
                                allgather(li_next, 0)
                        else:
                            break
                    if t == NTILE - 1:
                        while done[0] < len(allgroups):
                            g0, w = allgroups[done[0]]
                            emit_group(g0, w)
                            done[0] += 1
                            if done[0] == 7:
                                allgather(li_next, 0)
                        allgather(li_next, 1)

                return post

            # ---------- Layer 1 aggregation ----------
            def l2_group(g0, w):
                ph = psB.tile([128, 512], dt.float32, tag="mm")
                nc.tensor.matmul(ph[0:D2, 0:w], w2_sb[0:D1, :], out1T[0:D1, g0 : g0 + w])
                stage_table(1, ph, D2, g0, w, src_off=0)

            pass_h(0, 0, epA)
            pass_h(0, 1, mk_epB(0, D1, out1T, b_sb[0]), post_tile=mk_post(l2_group, 1))

            # ---------- Layer 2 aggregation ----------
            def l3_group(g0, w):
                stage_table(2, out2T, D2, g0, w)

            pass_h(1, 0, epA)
            pass_h(1, 1, mk_epB(1, D2, out2T, b_sb[1]), post_tile=mk_post(l3_group, 2))

            # ---------- Layer 3 aggregation (aggregate-first) ----------
            pass_h(2, 0, epA)
            pass_h(2, 1, mk_epB(2, D2, out3T, b_sb[2], final3=True))

            nc.sync.dma_start(out_d[:, :], out3T[0:D3, :])

    nc.compile()
    return nc


def kernel(**inputs):
    global LAST_EXEC_NS
    x = np.asarray(inputs["x"], np.float32)
    ei = np.asarray(inputs["edge_index"])
    W = [np.asarray(inputs[f"W{i}"], np.float32) for i in (1, 2, 3)]
    b = [np.asarray(inputs[f"b{i}"], np.float32) for i in (1, 2, 3)]

    ds, runlen, idxw, droww = _prep_graph(ei)
    nchunk_max = int((runlen // 128).max())
    key = (tuple(runlen.ravel().tolist()), nchunk_max)
    if key not in _CACHE:
        _CACHE[key] = _build(runlen, nchunk_max)
    nc = _CACHE[key]

    w1p = np.zeros((K1 * 128, D1), np.float32)
    w1p[:D0] = W[0]
    w1p = np.ascontiguousarray(w1p.reshape(K1, 128, D1)).astype(BF16)
    bp = []
    for i, d in enumerate((D1, D2, D3)):
        a = np.zeros((128, 1), np.float32)
        a[:d, 0] = b[i]
        bp.append(a)
    iota = np.tile(np.arange(128, dtype=np.float32), (128, 1)).astype(BF16)

    in_maps = []
    for c in range(NCORES):
        sl = slice(c * PER, (c + 1) * PER)
        xTp = np.zeros((K1 * 128, PAD), BF16)
        xTp[:D0, :PER] = x[sl].T.astype(BF16)
        dsl = np.zeros(PAD, np.float32)
        dsl[:PER] = ds[sl]
        in_maps.append(
            {
                "xT": xTp,
                "w1": w1p,
                "w2": W[1],
                "w3": W[2],
                "b1": bp[0],
                "b2": bp[1],
                "b3": bp[2],
                "dsrep": np.ascontiguousarray(np.broadcast_to(dsl, (128, PAD))),
                "iota": iota,
                "idxs": idxw[c],
                "dstrow": droww[c],
            }
        )

    trace = bool(int(os.environ.get("KERNEL_TRACE", "0")))
    if trace:
        try:
            import trnprof  # noqa: F401  (dev-only profiling shim)
        except ImportError:
            trace = False

    res = run_bass_kernel_spmd(nc, in_maps, list(range(NCORES)), trace=trace)
    LAST_EXEC_NS = res.exec_time_ns

    out = np.empty((N, D3), np.float32)
    for c in range(NCORES):
        out[c * PER : (c + 1) * PER] = res.results[c]["out"][:, :PER].T
    return out


# revision 26
# speedup vs baseline: 1.0659x; 1.0302x over previous
"""3-layer GCN (N=50000, E=1.6M + self-loops) on 8 TRN2 NeuronCores.

Node/data-parallel (per sharding hint): core c owns rows [6250c, 6250(c+1)),
padded to 6272. Per layer: local transform (feature-major on PE), deg^-1/2
prescale, PE transpose to node-major, and TWO AllGathers build replicated
bf16 feature tables (halves A/B, int16-addressable). Aggregation runs in two
passes per 128-dst tile: dma_gather (4 SWDGE queues) pulls the A-half edge
slots, PE computes aggT += M.T @ S (S = one-hot dst-row matrix, built once on
DVE via is_equal and spilled/reloaded through DRAM), partial sums park in an
SBUF accumulator while pass B overlaps the B-half AllGather. Epilogue applies
deg^-1/2 postscale + bias + ReLU; layer 3 aggregates first (associativity).
The next layer's transform/staging/AllGather-A are emitted interleaved with
pass B so collectives hide behind gather work.
"""
import os

import numpy as np
import ml_dtypes

import concourse.bacc as bacc
import concourse.bass as bass
import concourse.mybir as mybir
import concourse.tile as tile
from concourse.bass_utils import run_bass_kernel_spmd
from concourse.library_config import mlp
from concourse.masks import make_identity

N = 50000
NCORES = 8
PER = 6250
PAD = 6272            # per-core padded node count (49 * 128)
NTILE = PAD // 128    # 49
LOCA = 3200           # local split: half A = [0, 3200), half B = [3200, 6272)
LOCB = PAD - LOCA     # 3072
ROWSA = LOCA * NCORES  # 25600 (int16-safe)
ROWSB = LOCB * NCORES  # 24576
K1 = 12
D0, D1, D2, D3 = 1433, 100, 50, 7
ELEM = 128            # bf16 elements per table row (256B)
CALL_MAX = 1024       # SWDGE ring capacity per dma_gather call
BF16 = ml_dtypes.bfloat16

LAST_EXEC_NS = None
_CACHE = {}


def _ceil128(x):
    return (np.asarray(x) + 127) // 128 * 128


def _wrap_idx(a):
    w = a.reshape(-1, 16).T
    return np.tile(w, (8, 1)).astype(np.int16)


def _wrap_slot(a, dtype):
    return np.ascontiguousarray(a.reshape(-1, 128).T).astype(dtype)


def _prep_graph(edge_index):
    src = np.asarray(edge_index[0], np.int64)
    dst = np.asarray(edge_index[1], np.int64)
    loops = np.arange(N, dtype=np.int64)
    # deg counts self-loops (reference adds them); the loops themselves are
    # NOT gathered -- their ds[v]^2*H[v] term is added densely in pass A.
    deg = np.bincount(np.concatenate([dst, loops]), minlength=N).astype(np.float64)
    ds = (1.0 / np.sqrt(deg)).astype(np.float32)
    srcA = src
    dstA = dst

    core = dstA // PER
    local = dstA - core * PER
    tilep = local >> 7
    drow = (local & 127).astype(np.int64)
    score = srcA // PER
    slocal = srcA - score * PER
    half = (slocal >= LOCA).astype(np.int64)
    idx16 = np.where(half == 0, score * LOCA + slocal, score * LOCB + (slocal - LOCA))

    key = ((core * NTILE + tilep) * 2 + half).astype(np.int64)
    order = np.argsort(key, kind="stable")
    key_s = key[order]
    idx_s = idx16[order]
    drow_s = drow[order]

    ngroups = NCORES * NTILE * 2
    counts = np.bincount(key_s, minlength=ngroups).reshape(NCORES, NTILE, 2)
    runlen = _ceil128(counts.max(axis=0))
    runlen = np.maximum(runlen, 128)
    starts = np.zeros(ngroups + 1, np.int64)
    np.cumsum(np.bincount(key_s, minlength=ngroups), out=starts[1:])

    # slot layout: ALL A-runs (tile-major), then ALL B-runs
    stot = int(runlen.sum())
    offA = np.concatenate([[0], np.cumsum(runlen[:, 0])])
    offB = np.concatenate([[0], np.cumsum(runlen[:, 1])]) + offA[-1]
    idx_pad = np.zeros((NCORES, stot), np.int64)
    drow_pad = np.full((NCORES, stot), 300, np.int64)
    for t in range(NTILE):
        for h in range(2):
            o0 = int(offA[t] if h == 0 else offB[t])
            for c in range(NCORES):
                g = (c * NTILE + t) * 2 + h
                n = int(starts[g + 1] - starts[g])
                idx_pad[c, o0 : o0 + n] = idx_s[starts[g] : starts[g + 1]]
                drow_pad[c, o0 : o0 + n] = drow_s[starts[g] : starts[g + 1]]

    idxw = np.stack([_wrap_idx(idx_pad[c]) for c in range(NCORES)])
    droww = np.stack(
        [_wrap_slot(drow_pad[c].astype(np.float32), BF16) for c in range(NCORES)]
    )
    return ds, runlen, idxw, droww


def _build(runlen, nchunk_max):
    dt = mybir.dt
    stot = int(runlen.sum())
    nchunk_tot = stot // 128
    offA = np.concatenate([[0], np.cumsum(runlen[:, 0])])
    offB = np.concatenate([[0], np.cumsum(runlen[:, 1])]) + offA[-1]

    nc = bacc.Bacc("TRN2", target_bir_lowering=False, debug=False, num_swdge_queues=4)
    xT = nc.dram_tensor("xT", [K1 * 128, PAD], dt.bfloat16, kind="ExternalInput")
    w1 = nc.dram_tensor("w1", [K1, 128, D1], dt.bfloat16, kind="ExternalInput")
    w2 = nc.dram_tensor("w2", [D1, D2], dt.float32, kind="ExternalInput")
    w3 = nc.dram_tensor("w3", [D2, D3], dt.float32, kind="ExternalInput")
    b1 = nc.dram_tensor("b1", [128, 1], dt.float32, kind="ExternalInput")
    b2 = nc.dram_tensor("b2", [128, 1], dt.float32, kind="ExternalInput")
    b3 = nc.dram_tensor("b3", [128, 1], dt.float32, kind="ExternalInput")
    dsrep = nc.dram_tensor("dsrep", [128, PAD], dt.float32, kind="ExternalInput")
    iota = nc.dram_tensor("iota", [128, 128], dt.bfloat16, kind="ExternalInput")
    idxs = nc.dram_tensor("idxs", [128, stot // 16], dt.int16, kind="ExternalInput")
    dstrow = nc.dram_tensor("dstrow", [128, nchunk_tot], dt.bfloat16, kind="ExternalInput")
    out_d = nc.dram_tensor("out", [D3, PAD], dt.float32, kind="ExternalOutput")

    s_spill = nc.dram_tensor("s_spill", [128, nchunk_tot * 128], dt.bfloat16)
    ag_inA = [nc.dram_tensor(f"ag_inA{i}", [LOCA, ELEM], dt.bfloat16) for i in range(3)]
    ag_inB = [nc.dram_tensor(f"ag_inB{i}", [LOCB, ELEM], dt.bfloat16) for i in range(3)]
    tabA = [
        nc.dram_tensor(f"tabA{i}", [ROWSA, ELEM], dt.bfloat16, addr_space="Shared")
        for i in range(3)
    ]
    tabB = [
        nc.dram_tensor(f"tabB{i}", [ROWSB, ELEM], dt.bfloat16, addr_space="Shared")
        for i in range(3)
    ]

    # v-groups (128-multiples, not straddling LOCA)
    groups = [(g * 512, 512) for g in range(6)] + [(3072, 128)] + [
        (LOCA + g * 512, 512) for g in range(6)
    ]
    groupsA = groups[:7]
    groupsB = groups[7:]

    with tile.TileContext(nc) as tc:
        with (
            tc.tile_pool(name="const", bufs=1) as constp,
            tc.tile_pool(name="big", bufs=1) as bigp,
            tc.tile_pool(name="slab", bufs=1) as slabp,
            tc.tile_pool(name="stage", bufs=3) as stagep,
            tc.tile_pool(name="m", bufs=8) as mp,
            tc.tile_pool(name="s", bufs=4) as sp,
            tc.tile_pool(name="eptmp", bufs=2) as epp,
            tc.tile_pool(name="psA", bufs=2, space="PSUM") as psA,
            tc.tile_pool(name="psB", bufs=2, space="PSUM") as psB,
            tc.tile_pool(name="psC", bufs=2, space="PSUM") as psC,
        ):
            nc.gpsimd.load_library(mlp)
            ident = constp.tile([128, 128], dt.float32)
            make_identity(nc, ident[:])
            w1_sb = constp.tile([128, K1, D1], dt.bfloat16)
            nc.sync.dma_start(
                w1_sb[:],
                bass.AP(w1.ap().tensor, 0, [[D1, 128], [128 * D1, K1], [1, D1]]),
            )
            w2_sb = constp.tile([128, D2], dt.float32)
            nc.sync.dma_start(w2_sb[0:D1, :], w2[:, :])
            w3_sb = constp.tile([128, D3], dt.float32)
            nc.sync.dma_start(w3_sb[0:D2, :], w3[:, :])
            b_sb = []
            for bt in (b1, b2, b3):
                b = constp.tile([128, 1], dt.float32)
                nc.sync.dma_start(b[:], bt[:, :])
                b_sb.append(b)
            dsr = constp.tile([128, PAD], dt.float32)
            nc.sync.dma_start(dsr[:], dsrep[:, :])
            iot = constp.tile([128, 128], dt.bfloat16)
            nc.sync.dma_start(iot[:], iota[:, :])
            idx_sb = constp.tile([128, stot // 16], dt.int16)
            nc.sync.dma_start(idx_sb[:], idxs[:, :])
            drow_sb = constp.tile([128, nchunk_tot], dt.bfloat16)
            nc.sync.dma_start(drow_sb[:], dstrow[:, :])

            out1T = bigp.tile([128, PAD], dt.float32)
            out2T = bigp.tile([128, PAD], dt.float32)
            out3T = bigp.tile([128, PAD], dt.float32)
            acc = bigp.tile([128, PAD], dt.bfloat16)  # pass-A partial sums
            pre_own = bigp.tile([128, PAD], dt.bfloat16)  # own prescaled rows^T

            qn = [0]
            ni_regs = {}

            def ni_reg(ni):
                if ni not in ni_regs:
                    ni_regs[ni] = nc.gpsimd.to_reg(ni)
                return ni_regs[ni]

            def stage_table(li, src_big, du, g0, w, src_off=None):
                o = g0 if src_off is None else src_off
                ts = epp.tile([128, 512], dt.float32, tag="ts")
                nc.vector.tensor_tensor(
                    ts[0:du, 0:w], src_big[0:du, o : o + w],
                    dsr[0:du, g0 : g0 + w], mybir.AluOpType.mult,
                )
                nc.vector.tensor_copy(pre_own[0:du, g0 : g0 + w], ts[0:du, 0:w])
                agd = ag_inA[li] if g0 < LOCA else ag_inB[li]
                r0 = g0 if g0 < LOCA else g0 - LOCA
                for s in range(w // 128):
                    pt = psC.tile([128, 128], dt.float32, tag="pt")
                    nc.tensor.transpose(
                        pt[:], ts[0:du, s * 128 : (s + 1) * 128], ident[0:du, :]
                    )
                    st = stagep.tile([128, ELEM], dt.bfloat16, tag="st")
                    nc.vector.tensor_copy(st[:], pt[:])
                    nc.sync.dma_start(agd[r0 + s * 128 : r0 + (s + 1) * 128, :], st[:])

            def allgather(li, h):
                src = ag_inA[li] if h == 0 else ag_inB[li]
                dst = tabA[li] if h == 0 else tabB[li]
                nc.gpsimd.collective_compute(
                    "AllGather",
                    mybir.AluOpType.bypass,
                    replica_groups=[list(range(NCORES))],
                    ins=[src.ap().opt()],
                    outs=[dst.ap().opt()],
                )

            WCH = CALL_MAX // 128  # chunks per gather window

            def seg_s(t, h, li, m, nch, j0):
                """S chunks for (tile, half) into s tile at chunk offset j0."""
                chunk0 = int(offA[t] if h == 0 else offB[t]) // 128
                s_t = sp.tile([128, nchunk_max, 128], dt.bfloat16, tag="s")
                sd = s_spill[:, chunk0 * 128 : (chunk0 + nch) * 128]
                if li == 0:
                    da = drow_sb[:, chunk0 : chunk0 + nch].to_broadcast([128, nch, 128])
                    ia = iot[:, :]
                    ia = bass.AP(ia.tensor, ia.offset, [ia.ap[0], [0, nch], ia.ap[1]])
                    nc.vector.tensor_tensor(
                        s_t[:, 0:nch, :], da, ia, mybir.AluOpType.is_equal
                    )
                    nc.sync.dma_start(sd, s_t[:, 0:nch, :])
                else:
                    nc.sync.dma_start(s_t[:, 0:nch, :], sd)
                return s_t

            def pass_h(li, h, epilogue, post_tile=None):
                src = (tabA[li] if h == 0 else tabB[li])[:, :]
                base = 0 if h == 0 else int(offA[-1])
                total = int(runlen[:, h].sum())
                wins = {}

                def get_win(w):
                    if w not in wins:
                        ni = min(CALL_MAX, total - w * CALL_MAX)
                        mw = mp.tile([128, WCH, ELEM], dt.bfloat16, tag="m")
                        s0 = base + w * CALL_MAX
                        nc.gpsimd.dma_gather(
                            mw[:, 0 : ni // 128, :],
                            src,
                            idx_sb[:, s0 // 16 : (s0 + ni) // 16],
                            ni,
                            ni_reg(ni),
                            ELEM,
                            queue_num=qn[0] % 4,
                        )
                        qn[0] += 1
                        wins[w] = mw
                    return wins[w]

                for t in range(NTILE):
                    rl = int(runlen[t, h])
                    nch = rl // 128
                    slot0 = int(offA[t] if h == 0 else offB[t])
                    s_t = seg_s(t, h, li, None, nch, 0)
                    ps = psA.tile([128, 128], dt.float32, tag="agg")
                    for j in range(nch):
                        rel = (slot0 - base) // 128 + j
                        mw = get_win(rel // WCH)
                        nc.tensor.matmul(
                            ps[:], mw[:, rel % WCH, :], s_t[:, j, :],
                            start=(j == 0), stop=(j == nch - 1),
                        )
                    epilogue(t, ps)
                    if post_tile is not None:
                        post_tile(t)

            def epA(t, ps):
                # park pass-A sum + the self-loop (diagonal) term
                nc.vector.tensor_tensor(
                    acc[:, t * 128 : (t + 1) * 128], ps[:],
                    pre_own[:, t * 128 : (t + 1) * 128], mybir.AluOpType.add,
                )

            def mk_epB(li, du, out_big, bias, final3=False):
                def ep(t, ps):
                    sc = epp.tile([128, 128], dt.float32, tag="sc")
                    # total = psB_pass + accA (bf16 partials)
                    nc.vector.tensor_tensor(
                        sc[0:du, :], ps[0:du, :],
                        acc[0:du, t * 128 : (t + 1) * 128], mybir.AluOpType.add,
                    )
                    nc.vector.tensor_tensor(
                        sc[0:du, :], sc[0:du, :],
                        dsr[0:du, t * 128 : (t + 1) * 128], mybir.AluOpType.mult,
                    )
                    if final3:
                        po = psB.tile([128, 128], dt.float32, tag="mm3")
                        nc.tensor.matmul(po[0:D3, :], w3_sb[0:D2, :], sc[0:du, :])
                        nc.scalar.activation(
                            out_big[0:D3, t * 128 : (t + 1) * 128], po[0:D3, :],
                            mybir.ActivationFunctionType.Relu, bias=bias[0:D3, :],
                        )
                    else:
                        nc.scalar.activation(
                            out_big[0:du, t * 128 : (t + 1) * 128], sc[0:du, :],
                            mybir.ActivationFunctionType.Relu, bias=bias[0:du, :],
                        )
                return ep

            # ---------- Layer 1 transform + staged AGs ----------
            def l1_group(g0, w):
                slab = slabp.tile([128, K1, 512], dt.bfloat16, tag="slab")
                nc.sync.dma_start(
                    slab[:, :, 0:w],
                    bass.AP(xT.ap().tensor, g0, [[PAD, 128], [128 * PAD, K1], [1, w]]),
                )
                ph = psB.tile([128, 512], dt.float32, tag="mm")
                for k in range(K1):
                    nc.tensor.matmul(
                        ph[0:D1, 0:w], w1_sb[:, k, :], slab[:, k, 0:w],
                        start=(k == 0), stop=(k == K1 - 1),
                    )
                stage_table(0, ph, D1, g0, w, src_off=0)

            for g0, w in groupsA:
                l1_group(g0, w)
            allgather(0, 0)
            for g0, w in groupsB:
                l1_group(g0, w)
            allgather(0, 1)

            # interleaved emission of next-layer transform during pass B
            def mk_post(emit_group, li_next):
                done = [0]
                allgroups = groups

                def post(t):
                    # after tile t, columns up to (t+1)*128 of the source are ready
                    ready = (t + 1) * 128
                    while done[0] < len(allgroups):
                        g0, w = allgroups[done[0]]
                        if g0 + w <= ready:
                            emit_group(g0, w)
                            done[0] += 1
                            if done[0] == 7:
                                allgather(li_next, 0)
                        else:
                            break
                    if t == NTILE - 1:
                        while done[0] < len(allgroups):
                            g0, w = allgroups[done[0]]
                            emit_group(g0, w)
                            done[0] += 1
                            if done[0] == 7:
                                allgather(li_next, 0)
                        allgather(li_next, 1)

                return post

            # ---------- Layer 1 aggregation ----------
            def l2_group(g0, w):
                ph = psB.tile([128, 512], dt.float32, tag="mm")
                nc.tensor.matmul(ph[0:D2, 0:w], w2_sb[0:D1, :], out1T[0:D1, g0 : g0 + w])
                stage_table(1, ph, D2, g0, w, src_off=0)

            pass_h(0, 0, epA)
            pass_h(0, 1, mk_epB(0, D1, out1T, b_sb[0]), post_tile=mk_post(l2_group, 1))

            # ---------- Layer 2 aggregation ----------
            def l3_group(g0, w):
                stage_table(2, out2T, D2, g0, w)

            pass_h(1, 0, epA)
            pass_h(1, 1, mk_epB(1, D2, out2T, b_sb[1]), post_tile=mk_post(l3_group, 2))

            # ---------- Layer 3 aggregation (aggregate-first) ----------
            pass_h(2, 0, epA)
            pass_h(2, 1, mk_epB(2, D2, out3T, b_sb[2], final3=True))

            nc.sync.dma_start(out_d[:, :], out3T[0:D3, :])

    nc.compile()
    return nc


def kernel(**inputs):
    global LAST_EXEC_NS
    x = np.asarray(inputs["x"], np.float32)
    ei = np.asarray(inputs["edge_index"])
    W = [np.asarray(inputs[f"W{i}"], np.float32) for i in (1, 2, 3)]
    b = [np.asarray(inputs[f"b{i}"], np.float32) for i in (1, 2, 3)]

    ds, runlen, idxw, droww = _prep_graph(ei)
    nchunk_max = int((runlen // 128).max())
    key = (tuple(runlen.ravel().tolist()), nchunk_max)
    if key not in _CACHE:
        _CACHE[key] = _build(runlen, nchunk_max)
    nc = _CACHE[key]

    w1p = np.zeros((K1 * 128, D1), np.float32)
    w1p[:D0] = W[0]
    w1p = np.ascontiguousarray(w1p.reshape(K1, 128, D1)).astype(BF16)
    bp = []
    for i, d in enumerate((D1, D2, D3)):
        a = np.zeros((128, 1), np.float32)
        a[:d, 0] = b[i]
        bp.append(a)
    iota = np.tile(np.arange(128, dtype=np.float32), (128, 1)).astype(BF16)

    in_maps = []
    for c in range(NCORES):
        sl = slice(c * PER, (c + 1) * PER)
        xTp = np.zeros((K1 * 128, PAD), BF16)
        xTp[:D0, :PER] = x[sl].T.astype(BF16)
        dsl = np.zeros(PAD, np.float32)
        dsl[:PER] = ds[sl]
        in_maps.append(
            {
                "xT": xTp,
                "w1": w1p,
                "w2": W[1],
                "w3": W[2],
                "b1": bp[0],
                "b2": bp[1],
                "b3": bp[2],
                "dsrep": np.ascontiguousarray(np.broadcast_to(dsl, (128, PAD))),
                "iota": iota,
                "idxs": idxw[c],
                "dstrow": droww[c],
            }
        )

    trace = bool(int(os.environ.get("KERNEL_TRACE", "0")))
    if trace:
        try:
            import trnprof  # noqa: F401  (dev-only profiling shim)
        except ImportError:
            trace = False

    res = run_bass_kernel_spmd(nc, in_maps, list(range(NCORES)), trace=trace)
    LAST_EXEC_NS = res.exec_time_ns

    out = np.empty((N, D3), np.float32)
    for c in range(NCORES):
        out[c * PER : (c + 1) * PER] = res.results[c]["out"][:, :PER].T
    return out


# revision 27
# speedup vs baseline: 1.0664x; 1.0005x over previous
"""3-layer GCN (N=50000, E=1.6M + self-loops) on 8 TRN2 NeuronCores.

Node/data-parallel (per sharding hint): core c owns rows [6250c, 6250(c+1)),
padded to 6272. Per layer: local transform (feature-major on PE), deg^-1/2
prescale, PE transpose to node-major, and TWO AllGathers build replicated
bf16 feature tables (halves A/B, int16-addressable). Aggregation runs in two
passes per 128-dst tile: dma_gather (4 SWDGE queues) pulls the A-half edge
slots, PE computes aggT += M.T @ S (S = one-hot dst-row matrix, built once on
DVE via is_equal and spilled/reloaded through DRAM), partial sums park in an
SBUF accumulator while pass B overlaps the B-half AllGather. Epilogue applies
deg^-1/2 postscale + bias + ReLU; layer 3 aggregates first (associativity).
The next layer's transform/staging/AllGather-A are emitted interleaved with
pass B so collectives hide behind gather work.
"""
import os

import numpy as np
import ml_dtypes

import concourse.bacc as bacc
import concourse.bass as bass
import concourse.mybir as mybir
import concourse.tile as tile
from concourse.bass_utils import run_bass_kernel_spmd
from concourse.library_config import mlp
from concourse.masks import make_identity

N = 50000
NCORES = 8
PER = 6250
PAD = 6272            # per-core padded node count (49 * 128)
NTILE = PAD // 128    # 49
LOCA = 3200           # local split: half A = [0, 3200), half B = [3200, 6272)
LOCB = PAD - LOCA     # 3072
ROWSA = LOCA * NCORES  # 25600 (int16-safe)
ROWSB = LOCB * NCORES  # 24576
K1 = 12
D0, D1, D2, D3 = 1433, 100, 50, 7
ELEM = 128            # bf16 elements per table row (256B)
CALL_MAX = 896        # SWDGE ring capacity per dma_gather call
BF16 = ml_dtypes.bfloat16

LAST_EXEC_NS = None
_CACHE = {}


def _ceil128(x):
    return (np.asarray(x) + 127) // 128 * 128


def _wrap_idx(a):
    w = a.reshape(-1, 16).T
    return np.tile(w, (8, 1)).astype(np.int16)


def _wrap_slot(a, dtype):
    return np.ascontiguousarray(a.reshape(-1, 128).T).astype(dtype)


def _prep_graph(edge_index):
    src = np.asarray(edge_index[0], np.int64)
    dst = np.asarray(edge_index[1], np.int64)
    loops = np.arange(N, dtype=np.int64)
    # deg counts self-loops (reference adds them); the loops themselves are
    # NOT gathered -- their ds[v]^2*H[v] term is added densely in pass A.
    deg = np.bincount(np.concatenate([dst, loops]), minlength=N).astype(np.float64)
    ds = (1.0 / np.sqrt(deg)).astype(np.float32)
    srcA = src
    dstA = dst

    core = dstA // PER
    local = dstA - core * PER
    tilep = local >> 7
    drow = (local & 127).astype(np.int64)
    score = srcA // PER
    slocal = srcA - score * PER
    half = (slocal >= LOCA).astype(np.int64)
    idx16 = np.where(half == 0, score * LOCA + slocal, score * LOCB + (slocal - LOCA))

    key = ((core * NTILE + tilep) * 2 + half).astype(np.int64)
    order = np.argsort(key, kind="stable")
    key_s = key[order]
    idx_s = idx16[order]
    drow_s = drow[order]

    ngroups = NCORES * NTILE * 2
    counts = np.bincount(key_s, minlength=ngroups).reshape(NCORES, NTILE, 2)
    runlen = _ceil128(counts.max(axis=0))
    runlen = np.maximum(runlen, 128)
    starts = np.zeros(ngroups + 1, np.int64)
    np.cumsum(np.bincount(key_s, minlength=ngroups), out=starts[1:])

    # slot layout: ALL A-runs (tile-major), then ALL B-runs
    stot = int(runlen.sum())
    offA = np.concatenate([[0], np.cumsum(runlen[:, 0])])
    offB = np.concatenate([[0], np.cumsum(runlen[:, 1])]) + offA[-1]
    idx_pad = np.zeros((NCORES, stot), np.int64)
    drow_pad = np.full((NCORES, stot), 300, np.int64)
    for t in range(NTILE):
        for h in range(2):
            o0 = int(offA[t] if h == 0 else offB[t])
            for c in range(NCORES):
                g = (c * NTILE + t) * 2 + h
                n = int(starts[g + 1] - starts[g])
                idx_pad[c, o0 : o0 + n] = idx_s[starts[g] : starts[g + 1]]
                drow_pad[c, o0 : o0 + n] = drow_s[starts[g] : starts[g + 1]]

    idxw = np.stack([_wrap_idx(idx_pad[c]) for c in range(NCORES)])
    droww = np.stack(
        [_wrap_slot(drow_pad[c].astype(np.float32), BF16) for c in range(NCORES)]
    )
    return ds, runlen, idxw, droww


def _build(runlen, nchunk_max):
    dt = mybir.dt
    stot = int(runlen.sum())
    nchunk_tot = stot // 128
    offA = np.concatenate([[0], np.cumsum(runlen[:, 0])])
    offB = np.concatenate([[0], np.cumsum(runlen[:, 1])]) + offA[-1]

    nc = bacc.Bacc("TRN2", target_bir_lowering=False, debug=False, num_swdge_queues=4)
    xT = nc.dram_tensor("xT", [K1 * 128, PAD], dt.bfloat16, kind="ExternalInput")
    w1 = nc.dram_tensor("w1", [K1, 128, D1], dt.bfloat16, kind="ExternalInput")
    w2 = nc.dram_tensor("w2", [D1, D2], dt.float32, kind="ExternalInput")
    w3 = nc.dram_tensor("w3", [D2, D3], dt.float32, kind="ExternalInput")
    b1 = nc.dram_tensor("b1", [128, 1], dt.float32, kind="ExternalInput")
    b2 = nc.dram_tensor("b2", [128, 1], dt.float32, kind="ExternalInput")
    b3 = nc.dram_tensor("b3", [128, 1], dt.float32, kind="ExternalInput")
    dsrep = nc.dram_tensor("dsrep", [128, PAD], dt.float32, kind="ExternalInput")
    iota = nc.dram_tensor("iota", [128, 128], dt.bfloat16, kind="ExternalInput")
    idxs = nc.dram_tensor("idxs", [128, stot // 16], dt.int16, kind="ExternalInput")
    dstrow = nc.dram_tensor("dstrow", [128, nchunk_tot], dt.bfloat16, kind="ExternalInput")
    out_d = nc.dram_tensor("out", [D3, PAD], dt.float32, kind="ExternalOutput")

    s_spill = nc.dram_tensor("s_spill", [128, nchunk_tot * 128], dt.bfloat16)
    ag_inA = [nc.dram_tensor(f"ag_inA{i}", [LOCA, ELEM], dt.bfloat16) for i in range(3)]
    ag_inB = [nc.dram_tensor(f"ag_inB{i}", [LOCB, ELEM], dt.bfloat16) for i in range(3)]
    tabA = [
        nc.dram_tensor(f"tabA{i}", [ROWSA, ELEM], dt.bfloat16, addr_space="Shared")
        for i in range(3)
    ]
    tabB = [
        nc.dram_tensor(f"tabB{i}", [ROWSB, ELEM], dt.bfloat16, addr_space="Shared")
        for i in range(3)
    ]

    # v-groups (128-multiples, not straddling LOCA)
    groups = [(g * 512, 512) for g in range(6)] + [(3072, 128)] + [
        (LOCA + g * 512, 512) for g in range(6)
    ]
    groupsA = groups[:7]
    groupsB = groups[7:]

    with tile.TileContext(nc) as tc:
        with (
            tc.tile_pool(name="const", bufs=1) as constp,
            tc.tile_pool(name="big", bufs=1) as bigp,
            tc.tile_pool(name="slab", bufs=1) as slabp,
            tc.tile_pool(name="stage", bufs=3) as stagep,
            tc.tile_pool(name="m", bufs=8) as mp,
            tc.tile_pool(name="s", bufs=4) as sp,
            tc.tile_pool(name="eptmp", bufs=2) as epp,
            tc.tile_pool(name="psA", bufs=2, space="PSUM") as psA,
            tc.tile_pool(name="psB", bufs=2, space="PSUM") as psB,
            tc.tile_pool(name="psC", bufs=2, space="PSUM") as psC,
        ):
            nc.gpsimd.load_library(mlp)
            ident = constp.tile([128, 128], dt.float32)
            make_identity(nc, ident[:])
            w1_sb = constp.tile([128, K1, D1], dt.bfloat16)
            nc.sync.dma_start(
                w1_sb[:],
                bass.AP(w1.ap().tensor, 0, [[D1, 128], [128 * D1, K1], [1, D1]]),
            )
            w2_sb = constp.tile([128, D2], dt.float32)
            nc.sync.dma_start(w2_sb[0:D1, :], w2[:, :])
            w3_sb = constp.tile([128, D3], dt.float32)
            nc.sync.dma_start(w3_sb[0:D2, :], w3[:, :])
            b_sb = []
            for bt in (b1, b2, b3):
                b = constp.tile([128, 1], dt.float32)
                nc.sync.dma_start(b[:], bt[:, :])
                b_sb.append(b)
            dsr = constp.tile([128, PAD], dt.float32)
            nc.sync.dma_start(dsr[:], dsrep[:, :])
            iot = constp.tile([128, 128], dt.bfloat16)
            nc.sync.dma_start(iot[:], iota[:, :])
            idx_sb = constp.tile([128, stot // 16], dt.int16)
            nc.sync.dma_start(idx_sb[:], idxs[:, :])
            drow_sb = constp.tile([128, nchunk_tot], dt.bfloat16)
            nc.sync.dma_start(drow_sb[:], dstrow[:, :])

            out1T = bigp.tile([128, PAD], dt.float32)
            out2T = bigp.tile([128, PAD], dt.float32)
            out3T = bigp.tile([128, PAD], dt.float32)
            acc = bigp.tile([128, PAD], dt.bfloat16)  # pass-A partial sums
            pre_own = bigp.tile([128, PAD], dt.bfloat16)  # own prescaled rows^T

            qn = [0]
            ni_regs = {}

            def ni_reg(ni):
                if ni not in ni_regs:
                    ni_regs[ni] = nc.gpsimd.to_reg(ni)
                return ni_regs[ni]

            def stage_table(li, src_big, du, g0, w, src_off=None):
                o = g0 if src_off is None else src_off
                ts = epp.tile([128, 512], dt.float32, tag="ts")
                nc.vector.tensor_tensor(
                    ts[0:du, 0:w], src_big[0:du, o : o + w],
                    dsr[0:du, g0 : g0 + w], mybir.AluOpType.mult,
                )
                nc.vector.tensor_copy(pre_own[0:du, g0 : g0 + w], ts[0:du, 0:w])
                agd = ag_inA[li] if g0 < LOCA else ag_inB[li]
                r0 = g0 if g0 < LOCA else g0 - LOCA
                for s in range(w // 128):
                    pt = psC.tile([128, 128], dt.float32, tag="pt")
                    nc.tensor.transpose(
                        pt[:], ts[0:du, s * 128 : (s + 1) * 128], ident[0:du, :]
                    )
                    st = stagep.tile([128, ELEM], dt.bfloat16, tag="st")
                    nc.vector.tensor_copy(st[:], pt[:])
                    nc.sync.dma_start(agd[r0 + s * 128 : r0 + (s + 1) * 128, :], st[:])

            def allgather(li, h):
                src = ag_inA[li] if h == 0 else ag_inB[li]
                dst = tabA[li] if h == 0 else tabB[li]
                nc.gpsimd.collective_compute(
                    "AllGather",
                    mybir.AluOpType.bypass,
                    replica_groups=[list(range(NCORES))],
                    ins=[src.ap().opt()],
                    outs=[dst.ap().opt()],
                )

            WCH = CALL_MAX // 128  # chunks per gather window

            def seg_s(t, h, li, m, nch, j0):
                """S chunks for (tile, half) into s tile at chunk offset j0."""
                chunk0 = int(offA[t] if h == 0 else offB[t]) // 128
                s_t = sp.tile([128, nchunk_max, 128], dt.bfloat16, tag="s")
                sd = s_spill[:, chunk0 * 128 : (chunk0 + nch) * 128]
                if li == 0:
                    da = drow_sb[:, chunk0 : chunk0 + nch].to_broadcast([128, nch, 128])
                    ia = iot[:, :]
                    ia = bass.AP(ia.tensor, ia.offset, [ia.ap[0], [0, nch], ia.ap[1]])
                    nc.vector.tensor_tensor(
                        s_t[:, 0:nch, :], da, ia, mybir.AluOpType.is_equal
                    )
                    nc.sync.dma_start(sd, s_t[:, 0:nch, :])
                else:
                    nc.sync.dma_start(s_t[:, 0:nch, :], sd)
                return s_t

            def pass_h(li, h, epilogue, post_tile=None):
                src = (tabA[li] if h == 0 else tabB[li])[:, :]
                base = 0 if h == 0 else int(offA[-1])
                total = int(runlen[:, h].sum())
                wins = {}

                def get_win(w):
                    if w not in wins:
                        ni = min(CALL_MAX, total - w * CALL_MAX)
                        mw = mp.tile([128, WCH, ELEM], dt.bfloat16, tag="m")
                        s0 = base + w * CALL_MAX
                        nc.gpsimd.dma_gather(
                            mw[:, 0 : ni // 128, :],
                            src,
                            idx_sb[:, s0 // 16 : (s0 + ni) // 16],
                            ni,
                            ni_reg(ni),
                            ELEM,
                            queue_num=qn[0] % 4,
                        )
                        qn[0] += 1
                        wins[w] = mw
                    return wins[w]

                for t in range(NTILE):
                    rl = int(runlen[t, h])
                    nch = rl // 128
                    slot0 = int(offA[t] if h == 0 else offB[t])
                    s_t = seg_s(t, h, li, None, nch, 0)
                    ps = psA.tile([128, 128], dt.float32, tag="agg")
                    for j in range(nch):
                        rel = (slot0 - base) // 128 + j
                        mw = get_win(rel // WCH)
                        nc.tensor.matmul(
                            ps[:], mw[:, rel % WCH, :], s_t[:, j, :],
                            start=(j == 0), stop=(j == nch - 1),
                        )
                    epilogue(t, ps)
                    if post_tile is not None:
                        post_tile(t)

            def epA(t, ps):
                # park pass-A sum + the self-loop (diagonal) term
                nc.vector.tensor_tensor(
                    acc[:, t * 128 : (t + 1) * 128], ps[:],
                    pre_own[:, t * 128 : (t + 1) * 128], mybir.AluOpType.add,
                )

            def mk_epB(li, du, out_big, bias, final3=False):
                def ep(t, ps):
                    sc = epp.tile([128, 128], dt.float32, tag="sc")
                    # total = psB_pass + accA (bf16 partials)
                    nc.vector.tensor_tensor(
                        sc[0:du, :], ps[0:du, :],
                        acc[0:du, t * 128 : (t + 1) * 128], mybir.AluOpType.add,
                    )
                    nc.vector.tensor_tensor(
                        sc[0:du, :], sc[0:du, :],
                        dsr[0:du, t * 128 : (t + 1) * 128], mybir.AluOpType.mult,
                    )
                    if final3:
                        po = psB.tile([128, 128], dt.float32, tag="mm3")
                        nc.tensor.matmul(po[0:D3, :], w3_sb[0:D2, :], sc[0:du, :])
                        nc.scalar.activation(
                            out_big[0:D3, t * 128 : (t + 1) * 128], po[0:D3, :],
                            mybir.ActivationFunctionType.Relu, bias=bias[0:D3, :],
                        )
                    else:
                        nc.scalar.activation(
                            out_big[0:du, t * 128 : (t + 1) * 128], sc[0:du, :],
                            mybir.ActivationFunctionType.Relu, bias=bias[0:du, :],
                        )
                return ep

            # ---------- Layer 1 transform + staged AGs ----------
            def l1_group(g0, w):
                slab = slabp.tile([128, K1, 512], dt.bfloat16, tag="slab")
                nc.sync.dma_start(
                    slab[:, :, 0:w],
                    bass.AP(xT.ap().tensor, g0, [[PAD, 128], [128 * PAD, K1], [1, w]]),
                )
                ph = psB.tile([128, 512], dt.float32, tag="mm")
                for k in range(K1):
                    nc.tensor.matmul(
                        ph[0:D1, 0:w], w1_sb[:, k, :], slab[:, k, 0:w],
                        start=(k == 0), stop=(k == K1 - 1),
                    )
                stage_table(0, ph, D1, g0, w, src_off=0)

            for g0, w in groupsA:
                l1_group(g0, w)
            allgather(0, 0)
            for g0, w in groupsB:
                l1_group(g0, w)
            allgather(0, 1)

            # interleaved emission of next-layer transform during pass B
            def mk_post(emit_group, li_next):
                done = [0]
                allgroups = groups

                def post(t):
                    # after tile t, columns up to (t+1)*128 of the source are ready
                    ready = (t + 1) * 128
                    while done[0] < len(allgroups):
                        g0, w = allgroups[done[0]]
                        if g0 + w <= ready:
                            emit_group(g0, w)
                            done[0] += 1
                            if done[0] == 7:
                                allgather(li_next, 0)
                        else:
                            break
                    if t == NTILE - 1:
                        while done[0] < len(allgroups):
                            g0, w = allgroups[done[0]]
                            emit_group(g0, w)
                            done[0] += 1
                            if done[0] == 7:
                                allgather(li_next, 0)
                        allgather(li_next, 1)

                return post

            # ---------- Layer 1 aggregation ----------
            def l2_group(g0, w):
                ph = psB.tile([128, 512], dt.float32, tag="mm")
                nc.tensor.matmul(ph[0:D2, 0:w], w2_sb[0:D1, :], out1T[0:D1, g0 : g0 + w])
                stage_table(1, ph, D2, g0, w, src_off=0)

            pass_h(0, 0, epA)
            pass_h(0, 1, mk_epB(0, D1, out1T, b_sb[0]), post_tile=mk_post(l2_group, 1))

            # ---------- Layer 2 aggregation ----------
            def l3_group(g0, w):
                stage_table(2, out2T, D2, g0, w)

            pass_h(1, 0, epA)
            pass_h(1, 1, mk_epB(1, D2, out2T, b_sb[1]), post_tile=mk_post(l3_group, 2))

            # ---------- Layer 3 aggregation (aggregate-first) ----------
            pass_h(2, 0, epA)
            pass_h(2, 1, mk_epB(2, D2, out3T, b_sb[2], final3=True))

            nc.sync.dma_start(out_d[:, :], out3T[0:D3, :])

    nc.compile()
    return nc


def kernel(**inputs):
    global LAST_EXEC_NS
    x = np.asarray(inputs["x"], np.float32)
    ei = np.asarray(inputs["edge_index"])
    W = [np.asarray(inputs[f"W{i}"], np.float32) for i in (1, 2, 3)]
    b = [np.asarray(inputs[f"b{i}"], np.float32) for i in (1, 2, 3)]

    ds, runlen, idxw, droww = _prep_graph(ei)
    nchunk_max = int((runlen // 128).max())
    key = (tuple(runlen.ravel().tolist()), nchunk_max)
    if key not in _CACHE:
        _CACHE[key] = _build(runlen, nchunk_max)
    nc = _CACHE[key]

    w1p = np.zeros((K1 * 128, D1), np.float32)
    w1p[:D0] = W[0]
    w1p = np.ascontiguousarray(w1p.reshape(K1, 128, D1)).astype(BF16)
    bp = []
    for i, d in enumerate((D1, D2, D3)):
        a = np.zeros((128, 1), np.float32)
        a[:d, 0] = b[i]
        bp.append(a)
    iota = np.tile(np.arange(128, dtype=np.float32), (128, 1)).astype(BF16)

    in_maps = []
    for c in range(NCORES):
        sl = slice(c * PER, (c + 1) * PER)
        xTp = np.zeros((K1 * 128, PAD), BF16)
        xTp[:D0, :PER] = x[sl].T.astype(BF16)
        dsl = np.zeros(PAD, np.float32)
        dsl[:PER] = ds[sl]
        in_maps.append(
            {
                "xT": xTp,
                "w1": w1p,
                "w2": W[1],
                "w3": W[2],
                "b1": bp[0],
                "b2": bp[1],
                "b3": bp[2],
                "dsrep": np.ascontiguousarray(np.broadcast_to(dsl, (128, PAD))),
                "iota": iota,
                "idxs": idxw[c],
                "dstrow": droww[c],
            }
        )

    trace = bool(int(os.environ.get("KERNEL_TRACE", "0")))
    if trace:
        try:
            import trnprof  # noqa: F401  (dev-only profiling shim)
        except ImportError:
            trace = False

    res = run_bass_kernel_spmd(nc, in_maps, list(range(NCORES)), trace=trace)
    LAST_EXEC_NS = res.exec_time_ns

    out = np.empty((N, D3), np.float32)
    for c in range(NCORES):
        out[c * PER : (c + 1) * PER] = res.results[c]["out"][:, :PER].T
    return out
